# revision 1
# baseline (speedup 1.0000x reference)
"""Trainium2 Bass kernel: scatter rho[b, i, j] -> out[b, fock_idx[i], fock_idx[j]].

Sharding: batch dim B across the 8 NeuronCores (pure data parallel). fock_idx is
known on the host at call time, so the scatter addressing is baked into the
compiled program as static DMA/compute access patterns.

Per-core algorithm (out is [D, D], zero except out[idx[i], idx[j]] = rho[i, j]):
  - The runtime hands the NEFF a zero-initialized ExternalOutput buffer (both
    the native run_neff path and the axon/bass2jax donation path guarantee
    this), so only rows/columns that receive data are written.
  - fock_idx (for the real problem) is strictly increasing and decomposes into
    runs of consecutive indices (32 runs of 32). Columns: each rho row is
    expanded into a [span]-wide row in SBUF with the runs placed at their
    target offsets and zeros in the gaps. Rows: each 128-row tile of rho is
    stored with one DMA per row-run to the matching block of out rows,
    touching only columns [c0, c1).
  - The W expansion buffers are memset once up front and reused cyclically:
    the gap columns stay zero across reuse because the per-tile copies only
    ever write the (fixed) data columns.
  - Expansion copies run on Vector (single runs) and GpSimd (pair-merged
    runs); stores alternate between the two HWDGE rings (SP and ACT) so
    DMA issue is not serialized on one sequencer; loads ride GpSimd/SWDGE
    with lookahead. Measured ~51 us/core on
    TRN2 (~12.5 MB of HBM traffic/core at the shared-stack rate, plus ~9 us
    of fixed NEFF preamble/teardown).
"""

import numpy as np

import concourse.bacc as bacc
import concourse.bass as bass
import concourse.mybir as mybir
from concourse import tile
from concourse.bass_utils import run_bass_kernel_spmd

N_CORES = 8
P = 128  # SBUF partitions
W_BUFS = 4
R_BUFS = 6


def _runs(dst, src):
    """Maximal runs where dst and src both advance by 1. Yields (d0, s0, len)."""
    out = []
    d0, s0, L = int(dst[0]), int(src[0]), 1
    for k in range(1, len(dst)):
        if int(dst[k]) == d0 + L and int(src[k]) == s0 + L:
            L += 1
        else:
            out.append((d0, s0, L))
            d0, s0, L = int(dst[k]), int(src[k]), 1
    out.append((d0, s0, L))
    return out


def _pair_runs(col_runs):
    """Group adjacent equal-length runs into stride-2 pairs.

    Returns a list of (dst0, src0, pair_dst_stride, pair_src_stride, n, L)
    where n is 1 or 2 repeats of an L-wide copy.
    """
    out = []
    k = 0
    while k < len(col_runs):
        d0, s0, L = col_runs[k]
        if k + 1 < len(col_runs) and col_runs[k + 1][2] == L:
            d1, s1, _ = col_runs[k + 1]
            out.append((d0, s0, d1 - d0, s1 - s0, 2, L))
            k += 2
        else:
            out.append((d0, s0, L, L, 1, L))
            k += 1
    return out


def _build(idx, D, n):
    """Build the per-core Bass program with idx baked in."""
    f32 = mybir.dt.float32

    # Column placement: process columns in sorted-index order so the SBUF row
    # image is written left to right; a run needs source columns contiguous too.
    order = np.argsort(idx, kind="stable")
    col_runs = _runs(idx[order], order)  # (dst_col, src_col, len)
    c0 = min(r[0] for r in col_runs)
    c1 = max(r[0] + r[2] for r in col_runs)
    span = c1 - c0

    # ~18/32 runs to Vector as singles; 14 to GpSimd as pair-merged copies.
    # (Scalar is kept free to issue half the store DMAs.)
    runs_v = [r for k, r in enumerate(col_runs) if k % 16 < 9]
    pairs_g = _pair_runs([r for k, r in enumerate(col_runs) if k % 16 >= 9])

    nc = bacc.Bacc("TRN2", target_bir_lowering=False, debug=False,
                   num_devices=N_CORES)
    rho = nc.dram_tensor("rho", [n, n], f32, kind="ExternalInput")
    out = nc.dram_tensor("out", [D, D], f32, kind="ExternalOutput")

    n_tiles = (n + P - 1) // P
    with tile.TileContext(nc) as tc:
        with (
            tc.tile_pool(name="rp", bufs=R_BUFS) as rp,
            tc.tile_pool(name="wp", bufs=1) as wp,
        ):
            # W expansion buffers, memset once and reused cyclically, with
            # only W0/W1 up front and the rest staggered behind early tiles.
            ws = [wp.tile([P, span], f32, name=f"W{k}") for k in range(W_BUFS)]
            memset_eng = [nc.vector if k % 2 == 0 else nc.gpsimd
                          for k in range(W_BUFS)]

            n_store = 0

            # Loads ride the third DMA issue path (GpSimd/SWDGE) so both
            # HWDGE rings are dedicated to store issue. Emit with lookahead
            # so Q7 issues load t+3 before it starts tile t's pair copies.
            LOOKAHEAD = 3
            Rts = []

            def issue_load(t):
                r0 = t * P
                rows = min(P, n - r0)
                Rt = rp.tile([P, n], f32, name="R")
                nc.gpsimd.dma_start(Rt[:rows, :], rho[r0:r0 + rows, :])
                Rts.append(Rt)

            for t in range(min(LOOKAHEAD, n_tiles)):
                issue_load(t)

            # Memsets come after the lookahead loads so GpSimd's W1 memset
            # does not delay load issue on the Q7.
            memset_eng[0].memset(ws[0][:], 0.0)
            memset_eng[1].memset(ws[1][:], 0.0)
            next_memset = 2

            for t in range(n_tiles):
                r0 = t * P
                rows = min(P, n - r0)
                R = Rts[t]

                W = ws[t % W_BUFS]
                for d0, s0, L in runs_v:
                    nc.vector.tensor_copy(
                        W[:rows, d0 - c0:d0 - c0 + L],
                        R[:rows, s0:s0 + L])
                for d0, s0, ds, ss, cnt, L in pairs_g:
                    dst = bass.AP(W.tensor, W.offset + (d0 - c0),
                                  [[W.ap[0][0], rows], [ds, cnt], [1, L]])
                    src = bass.AP(R.tensor, R.offset + s0,
                                  [[R.ap[0][0], rows], [ss, cnt], [1, L]])
                    nc.gpsimd.tensor_copy(dst, src)

                # Row runs within this tile: consecutive rho rows with
                # consecutive target rows share one store DMA, alternating
                # between the SP and ACT HWDGE rings.
                for dr, sr, L in _runs(idx[r0:r0 + rows], range(rows)):
                    ring = nc.sync if n_store % 2 == 0 else nc.scalar
                    n_store += 1
                    ring.dma_start(out[dr:dr + L, c0:c1], W[sr:sr + L, :])

                if t + LOOKAHEAD < n_tiles:
                    issue_load(t + LOOKAHEAD)

                # Stagger the remaining one-time memsets behind early tiles.
                while next_memset < W_BUFS and next_memset <= t + 2:
                    memset_eng[next_memset].memset(ws[next_memset][:], 0.0)
                    next_memset += 1
    nc.compile()
    return nc


def kernel(input_state, fock_idx, fock_dim):
    input_state = np.asarray(input_state)
    idx = np.asarray(fock_idx).astype(np.int64)
    D = int(fock_dim)
    B, n, _ = input_state.shape

    nc = _build(idx, D, n)

    out = np.empty((B, D, D), dtype=input_state.dtype)
    for start in range(0, B, N_CORES):
        stop = min(start + N_CORES, B)
        in_maps = [
            {"rho": np.ascontiguousarray(input_state[b], dtype=np.float32)}
            for b in range(start, stop)
        ]
        res = run_bass_kernel_spmd(nc, in_maps,
                                   core_ids=list(range(stop - start)))
        for k, b in enumerate(range(start, stop)):
            out[b] = res.results[k]["out"]
    return out



# revision 2
# speedup vs baseline: 1.1127x; 1.1127x over previous
"""Trainium2 Bass kernel: scatter rho[b, i, j] -> out[b, fock_idx[i], fock_idx[j]].

Sharding: batch dim B across the 8 NeuronCores (pure data parallel). fock_idx is
known on the host at call time, so the scatter addressing is baked into the
compiled program as static DMA/compute access patterns.

Per-core algorithm (out is [D, D], zero except out[idx[i], idx[j]] = rho[i, j]):
  - The runtime hands the NEFF a zero-initialized ExternalOutput buffer (both
    the native run_neff path and the axon/bass2jax donation path guarantee
    this), so only rows/columns that receive data are written.
  - fock_idx (for the real problem) is strictly increasing and decomposes into
    runs of consecutive indices (32 runs of 32). Columns: each rho row is
    expanded into a [span]-wide row in SBUF with the runs placed at their
    target offsets and zeros in the gaps. Rows: each row-run of rho rows is
    stored with one DMA to the matching block of out rows, touching only
    columns [c0, c1).
  - All DMA rides the two HWDGE rings (SP and ACT): loads and stores both.
    SWDGE (gpsimd descriptor generation) is avoided entirely so GpSimd is
    free for expansion copies. Loads are issued first on each ring so the
    wire ramps immediately; stores queue behind them and drain as soon as
    each group's expansion completes.
  - Tiles are processed in groups of 2 (256 rho rows): the group's two
    [128, span] images live adjacent in one W buffer, so each column-run
    expansion copy moves both tiles in a single 3-dim-AP instruction
    ([128 parts][2 tiles][run]), halving per-instruction overhead. Copies
    alternate between Vector and GpSimd. W buffers are memset once, just
    ahead of their group's copies.
"""

import numpy as np

import concourse.bacc as bacc
import concourse.bass as bass
import concourse.mybir as mybir
from concourse import tile
from concourse.bass_utils import run_bass_kernel_spmd

N_CORES = 8
P = 128  # SBUF partitions
GT = 2   # tiles per group


def _runs(dst, src):
    """Maximal runs where dst and src both advance by 1. Yields (d0, s0, len)."""
    out = []
    d0, s0, L = int(dst[0]), int(src[0]), 1
    for k in range(1, len(dst)):
        if int(dst[k]) == d0 + L and int(src[k]) == s0 + L:
            L += 1
        else:
            out.append((d0, s0, L))
            d0, s0, L = int(dst[k]), int(src[k]), 1
    out.append((d0, s0, L))
    return out


def _build(idx, D, n):
    """Build the per-core Bass program with idx baked in."""
    f32 = mybir.dt.float32

    # Column placement: process columns in sorted-index order so the SBUF row
    # image is written left to right; a run needs source columns contiguous too.
    order = np.argsort(idx, kind="stable")
    col_runs = _runs(idx[order], order)  # (dst_col, src_col, len)
    c0 = min(r[0] for r in col_runs)
    c1 = max(r[0] + r[2] for r in col_runs)
    span = c1 - c0

    nc = bacc.Bacc("TRN2", target_bir_lowering=False, debug=False,
                   num_devices=N_CORES)
    rho = nc.dram_tensor("rho", [n, n], f32, kind="ExternalInput")
    out = nc.dram_tensor("out", [D, D], f32, kind="ExternalOutput")

    n_tiles = (n + P - 1) // P
    n_groups = (n_tiles + GT - 1) // GT
    with tile.TileContext(nc) as tc:
        with (
            tc.tile_pool(name="rp", bufs=1) as rp,
            tc.tile_pool(name="wp", bufs=1) as wp,
        ):
            Rs = [rp.tile([P, GT * n], f32, name=f"R{g}")
                  for g in range(n_groups)]
            Ws = [wp.tile([P, GT * span], f32, name=f"W{g}")
                  for g in range(n_groups)]

            # All loads up front, alternating rings, so each HWDGE ring's
            # FIFO starts with load descriptors and the wire ramps at once.
            for g in range(n_groups):
                ring = nc.sync if g % 2 == 0 else nc.scalar
                for j in range(GT):
                    r0 = (g * GT + j) * P
                    rows = min(P, n - r0)
                    ring.dma_start(Rs[g][:rows, j * n:j * n + n],
                                   rho[r0:r0 + rows, :])

            n_store = 0
            for g in range(n_groups):
                W, R = Ws[g], Rs[g]

                # Memset this group's W just ahead of its copies; gaps stay
                # zero because copies only touch the data columns.
                nc.vector.memset(W[:, 0:span], 0.0)
                nc.gpsimd.memset(W[:, span:GT * span], 0.0)

                # One instruction per column run covers both tiles of the
                # group: [128 partitions][GT tiles][run width].
                for k, (d0, s0, L) in enumerate(col_runs):
                    eng = nc.vector if k % 2 == 0 else nc.gpsimd
                    dst = bass.AP(W.tensor, W.offset + (d0 - c0),
                                  [[W.ap[0][0], P], [span, GT], [1, L]])
                    src = bass.AP(R.tensor, R.offset + s0,
                                  [[R.ap[0][0], P], [n, GT], [1, L]])
                    eng.tensor_copy(dst, src)

                # Row runs: consecutive rho rows with consecutive target rows
                # share one store DMA, alternating between the two rings.
                for j in range(GT):
                    r0 = (g * GT + j) * P
                    rows = min(P, n - r0)
                    for dr, sr, L in _runs(idx[r0:r0 + rows], range(rows)):
                        ring = nc.sync if n_store % 2 == 0 else nc.scalar
                        n_store += 1
                        ring.dma_start(out[dr:dr + L, c0:c1],
                                       W[sr:sr + L, j * span:j * span + span])
    nc.compile()
    return nc


def kernel(input_state, fock_idx, fock_dim):
    input_state = np.asarray(input_state)
    idx = np.asarray(fock_idx).astype(np.int64)
    D = int(fock_dim)
    B, n, _ = input_state.shape

    nc = _build(idx, D, n)

    out = np.empty((B, D, D), dtype=input_state.dtype)
    for start in range(0, B, N_CORES):
        stop = min(start + N_CORES, B)
        in_maps = [
            {"rho": np.ascontiguousarray(input_state[b], dtype=np.float32)}
            for b in range(start, stop)
        ]
        res = run_bass_kernel_spmd(nc, in_maps,
                                   core_ids=list(range(stop - start)))
        for k, b in enumerate(range(start, stop)):
            out[b] = res.results[k]["out"]
    return out
